# revision 11
# baseline (speedup 1.0000x reference)
"""GQA attention layer (B=2,S=2048,D=2048,H=16,KV=4,HD=128) on 8 trn2 cores.

Sharding: core = (b, g) for b in {0,1} (batch), g in {0..3} (kv group).
Each core computes q-heads 4g..4g+3 + kv head g for batch b, producing a
partial o-projection [S, D]; the host sums the 4 partials per batch.

Per-core kernel: transposed layout (head_dim on partitions), bf16 matmuls
with fp32 accumulation, softmax without max-subtraction (logits bounded
after RMSNorm), causal block skipping.

Attention for chunks c>=1 runs the probs (pb) and V in fp8e4 with
DoubleRow-paired matmuls (2 k-tiles per PE pass at 0.5 cyc/row): softmax
averaging over >=512 keys suppresses the fp8 noise to <1e-3 on the output
(validated numerically; chunk 0 stays bf16 because early rows average few
keys). The fp8 exp is shifted by e^-1 (max logit ~5.4, e4m3 max 448);
numerator and denominator share the shift so normalization cancels it.
DoubleRow outputs are hardware-restricted to PSUM partitions 0:63, so PV
runs two passes (hd 0:63 then 64:127) reusing one PSUM bank, and the
normalize muls write att partitions 64:127 from partition-0:63 operands
(DVE partition-shifted writes, validated on HW).
"""
import numpy as np
import ml_dtypes

B, S, DM = 2, 2048, 2048
H, KV, HD = 16, 4, 128
G = H // KV
THETA = 10000.0
EPS = 1e-6

P = 128         # partitions
CH = 512        # s-chunk (matmul N)
NCH = S // CH   # 4
KT = DM // P    # 16 contraction tiles
NST = S // P    # 16 s-tiles

_CACHE = {}
# extra kwargs for run_bass_kernel_spmd (test harness sets trace/tmpdir here)
_RUN_KWARGS = {}


def _build_nc():
    from concourse import bacc, mybir
    import concourse.tile as tile
    from contextlib import ExitStack

    f32 = mybir.dt.float32
    bf16 = mybir.dt.bfloat16
    f8 = mybir.dt.float8e4
    Act = mybir.ActivationFunctionType
    DR = mybir.MatmulPerfMode.DoubleRow

    nc = bacc.Bacc()
    d_xt = nc.declare_dram_parameter("xt", [NCH, P, KT, CH], bf16, isOutput=False)
    d_wq = nc.declare_dram_parameter("wq4", [P, KT, G, HD], bf16, isOutput=False)
    d_wk = nc.declare_dram_parameter("wk1", [P, KT, HD], bf16, isOutput=False)
    d_wv = nc.declare_dram_parameter("wv1", [P, KT, HD], bf16, isOutput=False)
    d_wo = nc.declare_dram_parameter("wo4", [HD, G, DM], bf16, isOutput=False)
    d_qs = nc.declare_dram_parameter("qsc", [HD, 1], f32, isOutput=False)
    d_ks = nc.declare_dram_parameter("ksc", [HD, 1], f32, isOutput=False)
    d_cos = nc.declare_dram_parameter("cos_t", [P, S], f32, isOutput=False)
    d_sin = nc.declare_dram_parameter("sin_t", [P, S], f32, isOutput=False)
    d_psw = nc.declare_dram_parameter("psw", [P, P], bf16, isOutput=False)
    d_tri = nc.declare_dram_parameter("tri", [P, P], bf16, isOutput=False)
    d_trineg = nc.declare_dram_parameter("trineg", [P, P], bf16, isOutput=False)
    d_out = nc.declare_dram_parameter("o_part", [S, DM], f32, isOutput=True)

    with tile.TileContext(nc) as tc, ExitStack() as ctx:
        const = ctx.enter_context(tc.tile_pool(name="const", bufs=1))
        xin = ctx.enter_context(tc.tile_pool(name="xin", bufs=2))
        work = ctx.enter_context(tc.tile_pool(name="work", bufs=4))
        pbp = ctx.enter_context(tc.tile_pool(name="pbp", bufs=18))
        wnorm = ctx.enter_context(tc.tile_pool(name="wnorm", bufs=3))
        # PSUM: 8 banks: pa(2) sc(2) cs(1) att(1) acc(2)
        p_pa = ctx.enter_context(tc.tile_pool(name="p_pa", bufs=2, space="PSUM"))
        p_sc = ctx.enter_context(tc.tile_pool(name="p_sc", bufs=2, space="PSUM"))
        p_cs = ctx.enter_context(tc.tile_pool(name="p_cs", bufs=1, space="PSUM"))
        p_att = ctx.enter_context(tc.tile_pool(name="p_att", bufs=1, space="PSUM"))
        p_acc = ctx.enter_context(tc.tile_pool(name="p_acc", bufs=2, space="PSUM"))

        # ---- persistent SBUF / DMA schedule ----
        # SP ring: wk, xt0 (first half), wv, then later chunks' x tiles.
        # GpSimd ring: xt0 second half (startup parallelism).
        # ACT ring: rope tables for chunk 0, small consts, wq, rest, wo.
        wk_sb = const.tile([P, KT, HD], bf16, tag="wk_sb")
        nc.sync.dma_start(out=wk_sb, in_=d_wk[:])
        xt0 = [xin.tile([P, 4, CH], bf16, tag=f"xt_c{i}", name=f"xt0_{i}") for i in range(4)]
        for i in range(2):
            nc.sync.dma_start(out=xt0[i], in_=d_xt[0, :, 4 * i:4 * i + 4])
        for i in range(2, 4):
            nc.gpsimd.dma_start(out=xt0[i], in_=d_xt[0, :, 4 * i:4 * i + 4])
        wv_sb = const.tile([P, KT, HD], bf16, tag="wv_sb")
        nc.sync.dma_start(out=wv_sb, in_=d_wv[:])

        cos_c = [const.tile([P, CH], f32, tag=f"cos{c}", name=f"cos_c{c}") for c in range(4)]
        sin_c = [const.tile([P, CH], f32, tag=f"sin{c}", name=f"sin_c{c}") for c in range(4)]
        nc.scalar.dma_start(out=cos_c[0], in_=d_cos[:, 0:CH])
        nc.scalar.dma_start(out=sin_c[0], in_=d_sin[:, 0:CH])
        qsc_sb = const.tile([HD, 1], f32, tag="qsc_sb")
        nc.scalar.dma_start(out=qsc_sb, in_=d_qs[:])
        ksc_sb = const.tile([HD, 1], f32, tag="ksc_sb")
        nc.scalar.dma_start(out=ksc_sb, in_=d_ks[:])
        psw_sb = const.tile([P, P], bf16, tag="psw_sb")
        nc.scalar.dma_start(out=psw_sb, in_=d_psw[:])
        tri_sb = const.tile([P, P], bf16, tag="tri_sb")
        nc.scalar.dma_start(out=tri_sb, in_=d_tri[:])
        trineg_sb = const.tile([P, P], bf16, tag="trineg_sb")
        nc.scalar.dma_start(out=trineg_sb, in_=d_trineg[:])
        wq_sb = [const.tile([P, 4, G, HD], bf16, tag=f"wq_sb{i}", name=f"wq_sb{i}") for i in range(4)]
        for i in range(4):
            nc.scalar.dma_start(out=wq_sb[i], in_=d_wq[:, 4 * i:4 * i + 4])
        for c in range(1, 4):
            nc.scalar.dma_start(out=cos_c[c], in_=d_cos[:, c * CH:(c + 1) * CH])
            nc.scalar.dma_start(out=sin_c[c], in_=d_sin[:, c * CH:(c + 1) * CH])
        wo_sb = const.tile([P, G, DM], bf16, tag="wo_sb")
        nc.scalar.dma_start(out=wo_sb, in_=d_wo[:])

        ones_bb = const.tile([P, P], bf16, tag="ones_bb")
        nc.vector.memset(ones_bb, 1.0)
        ones8 = const.tile([P, 2, 64], f8, tag="ones8")
        nc.vector.memset(ones8, 1.0)
        eps_q = const.tile([P, 1], f32, tag="eps_q")
        nc.vector.memset(eps_q, float(HD * EPS))
        eps_k = const.tile([P, 1], f32, tag="eps_k")
        nc.vector.memset(eps_k, float(EPS))
        negone = const.tile([P, 1], f32, tag="negone")
        nc.vector.memset(negone, -1.0)

        # roped q heads / k / v / normalized att, persistent
        qro = [const.tile([P, S], bf16, tag=f"qro{h}", name=f"qro{h}") for h in range(G)]
        kro = const.tile([P, S], bf16, tag="kro")
        v_sb = const.tile([P, NST, HD], bf16, tag="v_sb")
        att_sb = [const.tile([P, S], bf16, tag=f"att{h}", name=f"att{h}") for h in range(G)]

        # ---- Phase A (projections+rmsnorm+rope) per chunk ----
        def emit_A(c):
            cs = slice(c * CH, (c + 1) * CH)
            if c == 0:
                xt_t = xt0
            else:
                xt_t = [xin.tile([P, 4, CH], bf16, tag=f"xt_c{i}", name=f"xt{c}_{i}")
                        for i in range(4)]
                for i in range(2):
                    nc.sync.dma_start(out=xt_t[i], in_=d_xt[c, :, 4 * i:4 * i + 4])
                for i in range(2, 4):
                    nc.gpsimd.dma_start(out=xt_t[i], in_=d_xt[c, :, 4 * i:4 * i + 4])

            # k first (smallest weights -> earliest start), then v, then q heads
            for h in (G, G + 1, 0, 1, 2, 3):
                if h == G + 1:
                    # v in natural [s, hd] layout
                    for st in range(4):
                        vps = p_pa.tile([P, HD], f32, tag="pa")
                        for kt in range(KT):
                            nc.tensor.matmul(
                                vps, lhsT=xt_t[kt // 4][:, kt % 4, st * P:(st + 1) * P],
                                rhs=wv_sb[:, kt],
                                start=(kt == 0), stop=(kt == KT - 1),
                            )
                        nc.vector.tensor_copy(v_sb[:, 4 * c + st, :], vps)
                    continue
                is_q = h < G
                ps_q = p_pa.tile([P, CH], f32, tag="pa")
                for kt in range(KT):
                    lhs = wq_sb[kt // 4][:, kt % 4, h, :] if is_q else wk_sb[:, kt, :]
                    nc.tensor.matmul(
                        ps_q, lhsT=lhs, rhs=xt_t[kt // 4][:, kt % 4],
                        start=(kt == 0), stop=(kt == KT - 1),
                    )
                # rmsnorm: sumsq over hd via ones-matmul (M=128 -> broadcast rows)
                qsq = wnorm.tile([P, CH], bf16, tag="qsq")
                nc.scalar.activation(out=qsq, in_=ps_q, func=Act.Square)
                ss = p_sc.tile([P, CH], f32, tag="sc")
                nc.tensor.matmul(ss, lhsT=ones_bb, rhs=qsq, start=True, stop=True)
                ln = wnorm.tile([P, CH], f32, tag="ln")
                if is_q:
                    # rn = 1/sqrt(sumsq + HD*eps) == rmsnorm_scale * HD^-0.5
                    nc.scalar.activation(out=ln, in_=ss, func=Act.Ln,
                                         scale=1.0, bias=eps_q)
                else:
                    nc.scalar.activation(out=ln, in_=ss, func=Act.Ln,
                                         scale=1.0 / HD, bias=eps_k)
                rn = wnorm.tile([P, CH], f32, tag="rn")
                nc.scalar.activation(out=rn, in_=ln, func=Act.Exp, scale=-0.5)
                qs = work.tile([P, CH], bf16, tag="qs")
                nc.vector.scalar_tensor_tensor(
                    out=qs, in0=ps_q, scalar=(qsc_sb if is_q else ksc_sb), in1=rn,
                    op0=mybir.AluOpType.mult, op1=mybir.AluOpType.mult)
                # rope: out = qs*cos + swap(qs)*sin_signed (swap via PE permute)
                rot = p_sc.tile([P, CH], f32, tag="sc")
                nc.tensor.matmul(rot, lhsT=psw_sb, rhs=qs, start=True, stop=True)
                t1 = work.tile([P, CH], f32, tag="t1")
                nc.vector.tensor_mul(t1, qs, cos_c[c])
                u = work.tile([P, CH], f32, tag="u")
                nc.vector.tensor_mul(u, rot, sin_c[c])
                dst = qro[h] if is_q else kro
                nc.vector.tensor_add(dst[:, cs], t1, u)

        # ---- Phase B (attention), one head at a time ----
        # PE work is emitted in same-dtype/perf-mode blocks (all bf16 scores,
        # then all fp8-DR csum, then fp8-DR PV passes) to avoid PE pipeline
        # churn from mode switching.
        def emit_att_head(c, h):
            cs_ = slice(c * CH, (c + 1) * CH)
            csum = p_cs.tile([64, CH], f32, tag="cs")
            if c == 0:
                pbds = []
                for t in range(4):
                    off = P * t
                    sc = p_sc.tile([P, CH], f32, tag="sc")
                    nc.tensor.matmul(
                        sc[:, off:], lhsT=kro[:, t * P:(t + 1) * P],
                        rhs=qro[h][:, off:CH], start=True, stop=True,
                    )
                    pb = pbp.tile([P, CH], bf16, tag="pbd")
                    pbds.append((pb, off))
                    nc.scalar.activation(out=pb[:, off:], in_=sc[:, off:], func=Act.Exp)
                    nc.vector.tensor_mul(pb[:, off:off + P], pb[:, off:off + P], tri_sb)
                attps = p_att.tile([P, CH], f32, tag="att")
                for t in range(4):
                    pb, off = pbds[t]
                    nc.tensor.matmul(attps[:, off:], lhsT=v_sb[:, t, :],
                                     rhs=pb[:, off:], start=(t == 0), stop=(t == 3),
                                     skip_group_check=True)
                for t in range(4):
                    pb, off = pbds[t]
                    nc.tensor.matmul(csum[:, off:], lhsT=ones_bb[:, 0:64],
                                     rhs=pb[:, off:], start=(t == 0), stop=(t == 3),
                                     skip_group_check=True)
            else:
                ntile = 4 * c + 4
                pbds = []
                for t in range(ntile):
                    j = t - 4 * c
                    off = P * j if j > 0 else 0
                    sc = p_sc.tile([P, CH], f32, tag="sc")
                    nc.tensor.matmul(
                        sc[:, off:], lhsT=kro[:, t * P:(t + 1) * P],
                        rhs=qro[h][:, c * CH + off:(c + 1) * CH],
                        start=True, stop=True,
                    )
                    pb = pbp.tile([P, CH], bf16, tag="pbd")
                    pbds.append((pb, off))
                    nc.scalar.activation(out=pb[:, off:], in_=sc[:, off:], func=Act.Exp)
                    if j >= 0:
                        nc.vector.tensor_mul(pb[:, off:off + P], pb[:, off:off + P], tri_sb)
                attps = p_att.tile([P, CH], f32, tag="att")
                for t in range(ntile):
                    pb, off = pbds[t]
                    nc.tensor.matmul(attps[:, off:], lhsT=v_sb[:, t, :],
                                     rhs=pb[:, off:], start=(t == 0), stop=(t == ntile - 1),
                                     skip_group_check=True)
                for t in range(ntile):
                    pb, off = pbds[t]
                    nc.tensor.matmul(csum[:, off:], lhsT=ones_bb[:, 0:64],
                                     rhs=pb[:, off:], start=(t == 0), stop=(t == ntile - 1),
                                     skip_group_check=True)
            # normalize: rcp on 64 partitions, duplicated to 128 via a
            # partition-shifted DVE copy, then one mul
            rcp = wnorm.tile([P, CH], f32, tag="rn")
            scr = wnorm.tile([64, CH], f32, tag="ln")
            nc.vector.reciprocal_approx_accurate(out=rcp[0:64, :], in_=csum, scratch=scr)
            nc.vector.tensor_copy(rcp[64:128, :], rcp[0:64, :])
            nc.vector.tensor_mul(att_sb[h][:, cs_], attps, rcp)

        # ---- Phase C (o-proj) for one s-tile ----
        def emit_oproj_st(st):
            for mc in range(NCH):
                ops = p_acc.tile([P, CH], f32, tag="acc")
                for h in range(G):
                    nc.tensor.matmul(
                        ops, lhsT=att_sb[h][:, st * P:(st + 1) * P],
                        rhs=wo_sb[:, h, mc * CH:(mc + 1) * CH],
                        start=(h == 0), stop=(h == G - 1),
                    )
                osb = work.tile([P, CH], f32, tag="osb")
                nc.vector.tensor_copy(osb, ops)
                eng = nc.sync if (st * NCH + mc) % 2 == 0 else nc.scalar
                eng.dma_start(
                    out=d_out[st * P:(st + 1) * P, mc * CH:(mc + 1) * CH], in_=osb)

        # wavefront: attention heads of chunk c interleave with o-proj
        # s-tiles of the previous attention chunk (PE fill work during the
        # ACT-bound exp stretches); cheap chunk 0 last for a short tail.
        # x-chunk DMAs are issued one phase ahead of use.
        emit_A(0)
        emit_A(1)
        for h in range(G):
            emit_att_head(1, h)
        emit_A(2)
        for h in range(G):
            emit_att_head(2, h)
            emit_oproj_st(4 * 1 + h)
        emit_A(3)
        for h in range(G):
            emit_att_head(3, h)
            emit_oproj_st(4 * 2 + h)
        for h in range(G):
            emit_att_head(0, h)
            emit_oproj_st(4 * 3 + h)
        for st in range(4):
            emit_oproj_st(st)

    # Pin every activation to the one table set that contains all functions
    # we use (exp/ln/copy/square), so the ACT engine never swaps tables.
    # Indices must stay aligned with act_info.json, so other sets are kept
    # in place but emptied (the pass then can't pick them).
    from concourse import bacc as bacc_mod
    orig_tables = bacc_mod.get_activation_tables
    target = "natural_log_exp_and_others"

    def unified_tables(arch):
        t = orig_tables(arch)
        assert target in t
        return {k: (v if k == target else set()) for k, v in t.items()}

    bacc_mod.get_activation_tables = unified_tables
    try:
        nc.compile()
    finally:
        bacc_mod.get_activation_tables = orig_tables
    return nc


def _get_nc():
    if "nc" not in _CACHE:
        _CACHE["nc"] = _build_nc()
    return _CACHE["nc"]


def _rope_tables():
    inv_ts = THETA ** (-np.arange(HD // 2, dtype=np.float64) / (HD // 2))
    ang = np.arange(S, dtype=np.float64)[None, :] * inv_ts[:, None]  # [64, S]
    cos64 = np.cos(ang)
    sin64 = np.sin(ang)
    cos_t = np.concatenate([cos64, cos64], 0).astype(np.float32)
    # rotate-then-multiply signs: top rows get -sin, bottom +sin
    sin_t = np.concatenate([-sin64, sin64], 0).astype(np.float32)
    return cos_t, sin_t


def kernel(x, wq, wk, wv, wo, q_scale, k_scale):
    bf = ml_dtypes.bfloat16
    x = np.asarray(x, np.float32)
    wq = np.asarray(wq, np.float32)
    wk = np.asarray(wk, np.float32)
    wv = np.asarray(wv, np.float32)
    wo = np.asarray(wo, np.float32)
    q_scale = np.asarray(q_scale, np.float32)
    k_scale = np.asarray(k_scale, np.float32)

    from concourse.bass_utils import run_bass_kernel_spmd

    nc = _get_nc()
    cos_t, sin_t = _rope_tables()
    half = P // 2
    psw = np.zeros((P, P), np.float32)
    psw[np.arange(half) + half, np.arange(half)] = 1.0
    psw[np.arange(half), np.arange(half) + half] = 1.0
    tri = (np.arange(P)[None, :] >= np.arange(P)[:, None]).astype(np.float32)
    trineg = np.where(np.arange(P)[None, :] >= np.arange(P)[:, None], 0.0, -50.0
                      ).astype(np.float32)

    in_maps = []
    for core in range(8):
        b, g = divmod(core, 4)
        in_maps.append({
            "xt": np.ascontiguousarray(
                x[b].T.reshape(KT, P, NCH, CH).transpose(2, 1, 0, 3)).astype(bf),
            "wq4": np.ascontiguousarray(
                wq[:, 4 * g:4 * g + 4, :].reshape(KT, P, G, HD).transpose(1, 0, 2, 3)).astype(bf),
            "wk1": np.ascontiguousarray(
                wk[:, g, :].reshape(KT, P, HD).transpose(1, 0, 2)).astype(bf),
            "wv1": np.ascontiguousarray(
                wv[:, g, :].reshape(KT, P, HD).transpose(1, 0, 2)).astype(bf),
            "wo4": np.ascontiguousarray(np.transpose(wo[4 * g:4 * g + 4], (1, 0, 2))).astype(bf),
            "qsc": q_scale.reshape(HD, 1),
            "ksc": k_scale.reshape(HD, 1),
            "cos_t": cos_t,
            "sin_t": sin_t,
            "psw": psw.astype(bf),
            "tri": tri.astype(bf),
            "trineg": trineg.astype(bf),
        })

    res = run_bass_kernel_spmd(nc, in_maps, list(range(8)), **_RUN_KWARGS)
    _CACHE["last_res"] = res
    out = np.zeros((B, S, DM), np.float32)
    for core in range(8):
        out[core // 4] += res.results[core]["o_part"]
    return out


# revision 12
# speedup vs baseline: 1.1065x; 1.1065x over previous
"""GQA attention layer (B=2,S=2048,D=2048,H=16,KV=4,HD=128) on 8 trn2 cores.

Sharding: core = (b, g) for b in {0,1} (batch), g in {0..3} (kv group).
Each core computes q-heads 4g..4g+3 + kv head g for batch b, producing a
partial o-projection [S, D]; the host sums the 4 partials per batch.

Per-core kernel: transposed layout (head_dim on partitions), bf16 matmuls
with fp32 accumulation, softmax without max-subtraction (logits bounded
after RMSNorm), causal block skipping.

Attention for chunks c>=1 runs the probs (pb) and V in fp8e4 with
DoubleRow-paired matmuls (2 k-tiles per PE pass at 0.5 cyc/row): softmax
averaging over >=512 keys suppresses the fp8 noise to <1e-3 on the output
(validated numerically; chunk 0 stays bf16 because early rows average few
keys). The fp8 exp is shifted by e^-1 (max logit ~5.4, e4m3 max 448);
numerator and denominator share the shift so normalization cancels it.
DoubleRow outputs are hardware-restricted to PSUM partitions 0:63, so PV
runs two passes (hd 0:63 then 64:127) reusing one PSUM bank, and the
normalize muls write att partitions 64:127 from partition-0:63 operands
(DVE partition-shifted writes, validated on HW).
"""
import numpy as np
import ml_dtypes

B, S, DM = 2, 2048, 2048
H, KV, HD = 16, 4, 128
G = H // KV
THETA = 10000.0
EPS = 1e-6

P = 128         # partitions
CH = 512        # s-chunk (matmul N)
NCH = S // CH   # 4
KT = DM // P    # 16 contraction tiles
NST = S // P    # 16 s-tiles

_CACHE = {}
# extra kwargs for run_bass_kernel_spmd (test harness sets trace/tmpdir here)
_RUN_KWARGS = {}


def _build_nc():
    from concourse import bacc, mybir
    import concourse.tile as tile
    from contextlib import ExitStack

    f32 = mybir.dt.float32
    bf16 = mybir.dt.bfloat16
    f8 = mybir.dt.float8e4
    Act = mybir.ActivationFunctionType
    DR = mybir.MatmulPerfMode.DoubleRow

    nc = bacc.Bacc()
    d_xt = nc.declare_dram_parameter("xt", [NCH, P, KT, CH], bf16, isOutput=False)
    d_wq = nc.declare_dram_parameter("wq4", [P, KT, G, HD], bf16, isOutput=False)
    d_wk = nc.declare_dram_parameter("wk1", [P, KT, HD], bf16, isOutput=False)
    d_wv = nc.declare_dram_parameter("wv1", [P, KT, HD], bf16, isOutput=False)
    d_wo = nc.declare_dram_parameter("wo4", [HD, G, DM], bf16, isOutput=False)
    d_qs = nc.declare_dram_parameter("qsc", [HD, 1], f32, isOutput=False)
    d_ks = nc.declare_dram_parameter("ksc", [HD, 1], f32, isOutput=False)
    d_cos = nc.declare_dram_parameter("cos_t", [P, S], f32, isOutput=False)
    d_sin = nc.declare_dram_parameter("sin_t", [P, S], f32, isOutput=False)
    d_psw = nc.declare_dram_parameter("psw", [P, P], bf16, isOutput=False)
    d_tri = nc.declare_dram_parameter("tri", [P, P], bf16, isOutput=False)
    d_trineg = nc.declare_dram_parameter("trineg", [P, P], bf16, isOutput=False)
    d_out = nc.declare_dram_parameter("o_part", [S, DM], f32, isOutput=True)

    with tile.TileContext(nc) as tc, ExitStack() as ctx:
        const = ctx.enter_context(tc.tile_pool(name="const", bufs=1))
        xin = ctx.enter_context(tc.tile_pool(name="xin", bufs=2))
        work = ctx.enter_context(tc.tile_pool(name="work", bufs=4))
        pbp = ctx.enter_context(tc.tile_pool(name="pbp", bufs=12))
        wnorm = ctx.enter_context(tc.tile_pool(name="wnorm", bufs=3))
        # PSUM: 8 banks: pa(2) sc(2) cs(1) att(1) acc(2)
        p_pa = ctx.enter_context(tc.tile_pool(name="p_pa", bufs=2, space="PSUM"))
        p_sc = ctx.enter_context(tc.tile_pool(name="p_sc", bufs=2, space="PSUM"))
        p_cs = ctx.enter_context(tc.tile_pool(name="p_cs", bufs=1, space="PSUM"))
        p_att = ctx.enter_context(tc.tile_pool(name="p_att", bufs=1, space="PSUM"))
        p_acc = ctx.enter_context(tc.tile_pool(name="p_acc", bufs=2, space="PSUM"))

        # ---- persistent SBUF / DMA schedule ----
        # SP ring: wk, xt0 (first half), wv, then later chunks' x tiles.
        # GpSimd ring: xt0 second half (startup parallelism).
        # ACT ring: rope tables for chunk 0, small consts, wq, rest, wo.
        wk_sb = const.tile([P, KT, HD], bf16, tag="wk_sb")
        nc.sync.dma_start(out=wk_sb, in_=d_wk[:])
        xt0 = [xin.tile([P, 4, CH], bf16, tag=f"xt_c{i}", name=f"xt0_{i}") for i in range(4)]
        for i in range(2):
            nc.sync.dma_start(out=xt0[i], in_=d_xt[0, :, 4 * i:4 * i + 4])
        for i in range(2, 4):
            nc.gpsimd.dma_start(out=xt0[i], in_=d_xt[0, :, 4 * i:4 * i + 4])
        wv_sb = const.tile([P, KT, HD], bf16, tag="wv_sb")
        nc.sync.dma_start(out=wv_sb, in_=d_wv[:])

        cos_c = [const.tile([P, CH], f32, tag=f"cos{c}", name=f"cos_c{c}") for c in range(4)]
        sin_c = [const.tile([P, CH], f32, tag=f"sin{c}", name=f"sin_c{c}") for c in range(4)]
        nc.scalar.dma_start(out=cos_c[0], in_=d_cos[:, 0:CH])
        nc.scalar.dma_start(out=sin_c[0], in_=d_sin[:, 0:CH])
        qsc_sb = const.tile([HD, 1], f32, tag="qsc_sb")
        nc.scalar.dma_start(out=qsc_sb, in_=d_qs[:])
        ksc_sb = const.tile([HD, 1], f32, tag="ksc_sb")
        nc.scalar.dma_start(out=ksc_sb, in_=d_ks[:])
        psw_sb = const.tile([P, P], bf16, tag="psw_sb")
        nc.scalar.dma_start(out=psw_sb, in_=d_psw[:])
        tri_sb = const.tile([P, P], bf16, tag="tri_sb")
        nc.scalar.dma_start(out=tri_sb, in_=d_tri[:])
        trineg_sb = const.tile([P, P], bf16, tag="trineg_sb")
        nc.scalar.dma_start(out=trineg_sb, in_=d_trineg[:])
        wq_sb = [const.tile([P, 4, G, HD], bf16, tag=f"wq_sb{i}", name=f"wq_sb{i}") for i in range(4)]
        for i in range(4):
            nc.scalar.dma_start(out=wq_sb[i], in_=d_wq[:, 4 * i:4 * i + 4])
        for c in range(1, 4):
            nc.scalar.dma_start(out=cos_c[c], in_=d_cos[:, c * CH:(c + 1) * CH])
            nc.scalar.dma_start(out=sin_c[c], in_=d_sin[:, c * CH:(c + 1) * CH])
        wo_sb = const.tile([P, G, DM], bf16, tag="wo_sb")
        nc.scalar.dma_start(out=wo_sb, in_=d_wo[:])

        ones_bb = const.tile([P, P], bf16, tag="ones_bb")
        nc.vector.memset(ones_bb, 1.0)
        ones8 = const.tile([P, 2, 64], f8, tag="ones8")
        nc.vector.memset(ones8, 1.0)
        eps_q = const.tile([P, 1], f32, tag="eps_q")
        nc.vector.memset(eps_q, float(HD * EPS))
        eps_k = const.tile([P, 1], f32, tag="eps_k")
        nc.vector.memset(eps_k, float(EPS))
        negone = const.tile([P, 1], f32, tag="negone")
        nc.vector.memset(negone, -1.0)

        # roped q heads / k / v / normalized att, persistent
        qro = [const.tile([P, S], bf16, tag=f"qro{h}", name=f"qro{h}") for h in range(G)]
        kro = const.tile([P, S], bf16, tag="kro")
        v_sb = const.tile([P, NST, HD], bf16, tag="v_sb")
        att_sb = [const.tile([P, S], bf16, tag=f"att{h}", name=f"att{h}") for h in range(G)]

        # ---- Phase A (projections+rmsnorm+rope) per chunk ----
        def emit_A(c):
            cs = slice(c * CH, (c + 1) * CH)
            if c == 0:
                xt_t = xt0
            else:
                xt_t = [xin.tile([P, 4, CH], bf16, tag=f"xt_c{i}", name=f"xt{c}_{i}")
                        for i in range(4)]
                for i in range(4):
                    nc.sync.dma_start(out=xt_t[i], in_=d_xt[c, :, 4 * i:4 * i + 4])

            # k first (smallest weights -> earliest start), then v, then q heads
            for h in (G, G + 1, 0, 1, 2, 3):
                if h == G + 1:
                    # v in natural [s, hd] layout
                    for st in range(4):
                        vps = p_pa.tile([P, HD], f32, tag="pa")
                        for kt in range(KT):
                            nc.tensor.matmul(
                                vps, lhsT=xt_t[kt // 4][:, kt % 4, st * P:(st + 1) * P],
                                rhs=wv_sb[:, kt],
                                start=(kt == 0), stop=(kt == KT - 1),
                            )
                        nc.vector.tensor_copy(v_sb[:, 4 * c + st, :], vps)
                    continue
                is_q = h < G
                ps_q = p_pa.tile([P, CH], f32, tag="pa")
                for kt in range(KT):
                    lhs = wq_sb[kt // 4][:, kt % 4, h, :] if is_q else wk_sb[:, kt, :]
                    nc.tensor.matmul(
                        ps_q, lhsT=lhs, rhs=xt_t[kt // 4][:, kt % 4],
                        start=(kt == 0), stop=(kt == KT - 1),
                    )
                # rmsnorm: sumsq over hd via ones-matmul (M=128 -> broadcast rows)
                qsq = wnorm.tile([P, CH], bf16, tag="qsq")
                nc.scalar.activation(out=qsq, in_=ps_q, func=Act.Square)
                ss = p_sc.tile([P, CH], f32, tag="sc")
                nc.tensor.matmul(ss, lhsT=ones_bb, rhs=qsq, start=True, stop=True)
                ln = wnorm.tile([P, CH], f32, tag="ln")
                if is_q:
                    # rn = 1/sqrt(sumsq + HD*eps) == rmsnorm_scale * HD^-0.5
                    nc.scalar.activation(out=ln, in_=ss, func=Act.Ln,
                                         scale=1.0, bias=eps_q)
                else:
                    nc.scalar.activation(out=ln, in_=ss, func=Act.Ln,
                                         scale=1.0 / HD, bias=eps_k)
                rn = wnorm.tile([P, CH], f32, tag="rn")
                nc.scalar.activation(out=rn, in_=ln, func=Act.Exp, scale=-0.5)
                qs = work.tile([P, CH], bf16, tag="qs")
                nc.vector.scalar_tensor_tensor(
                    out=qs, in0=ps_q, scalar=(qsc_sb if is_q else ksc_sb), in1=rn,
                    op0=mybir.AluOpType.mult, op1=mybir.AluOpType.mult)
                # rope: out = qs*cos + swap(qs)*sin_signed (swap via PE permute)
                rot = p_sc.tile([P, CH], f32, tag="sc")
                nc.tensor.matmul(rot, lhsT=psw_sb, rhs=qs, start=True, stop=True)
                t1 = work.tile([P, CH], f32, tag="t1")
                nc.vector.tensor_mul(t1, qs, cos_c[c])
                u = work.tile([P, CH], f32, tag="u")
                nc.vector.tensor_mul(u, rot, sin_c[c])
                dst = qro[h] if is_q else kro
                nc.vector.tensor_add(dst[:, cs], t1, u)

        # ---- Phase B (attention), one head at a time ----
        # PE work is emitted in same-dtype/perf-mode blocks (all bf16 scores,
        # then all fp8-DR csum, then fp8-DR PV passes) to avoid PE pipeline
        # churn from mode switching.
        def emit_att_head(c, h):
            cs_ = slice(c * CH, (c + 1) * CH)
            csum = p_cs.tile([64, CH], f32, tag="cs")
            if c == 0:
                pbds = []
                for t in range(4):
                    off = P * t
                    sc = p_sc.tile([P, CH], f32, tag="sc")
                    nc.tensor.matmul(
                        sc[:, off:], lhsT=kro[:, t * P:(t + 1) * P],
                        rhs=qro[h][:, off:CH], start=True, stop=True,
                    )
                    pb = pbp.tile([P, CH], bf16, tag="pbd")
                    pbds.append((pb, off))
                    nc.scalar.activation(out=pb[:, off:], in_=sc[:, off:], func=Act.Exp)
                    nc.vector.tensor_mul(pb[:, off:off + P], pb[:, off:off + P], tri_sb)
                for t in range(4):
                    pb, off = pbds[t]
                    nc.tensor.matmul(csum[:, off:], lhsT=ones_bb[:, 0:64],
                                     rhs=pb[:, off:], start=(t == 0), stop=(t == 3),
                                     skip_group_check=True)
                attps = p_att.tile([P, CH], f32, tag="att")
                for t in range(4):
                    pb, off = pbds[t]
                    nc.tensor.matmul(attps[:, off:], lhsT=v_sb[:, t, :],
                                     rhs=pb[:, off:], start=(t == 0), stop=(t == 3),
                                     skip_group_check=True)
            else:
                # scores into fp8 pair tiles (exp shifted by e^-1); csum via
                # fp8 DoubleRow over pairs (2 k-tiles per PE pass); PV stays
                # bf16 M=128 with causal off-skipping, reading fp8 pb slices.
                npair = 2 * c + 2
                pb2s = []
                for tp in range(npair):
                    pb2 = pbp.tile([P, 2, CH], f8, tag="pb2")
                    pb2s.append(pb2)
                    for i in (0, 1):
                        t = 2 * tp + i
                        j = t - 4 * c
                        sc = p_sc.tile([P, CH], f32, tag="sc")
                        if j < 0:
                            nc.tensor.matmul(
                                sc, lhsT=kro[:, t * P:(t + 1) * P],
                                rhs=qro[h][:, cs_], start=True, stop=True,
                            )
                            nc.scalar.activation(out=pb2[:, i, :], in_=sc,
                                                 func=Act.Exp, bias=negone)
                        else:
                            off = P * j
                            nc.tensor.matmul(
                                sc[:, off:], lhsT=kro[:, t * P:(t + 1) * P],
                                rhs=qro[h][:, c * CH + off:(c + 1) * CH],
                                start=True, stop=True,
                            )
                            nc.vector.tensor_add(sc[:, off:off + P],
                                                 sc[:, off:off + P], trineg_sb)
                            if off:
                                nc.vector.memset(pb2[:, i, 0:off], 0.0)
                            nc.scalar.activation(out=pb2[:, i, off:], in_=sc[:, off:],
                                                 func=Act.Exp, bias=negone)
                for tp in range(npair):
                    nc.tensor.matmul(csum, lhsT=ones8, rhs=pb2s[tp],
                                     start=(tp == 0), stop=(tp == npair - 1),
                                     perf_mode=DR, skip_group_check=True)
                attps = p_att.tile([P, CH], f32, tag="att")
                for t in range(4 * c + 4):
                    j = t - 4 * c
                    off = P * j if j > 0 else 0
                    nc.tensor.matmul(attps[:, off:], lhsT=v_sb[:, t, :],
                                     rhs=pb2s[t // 2][:, t % 2, off:],
                                     start=(t == 0), stop=(t == 4 * c + 3),
                                     skip_group_check=True)
            # normalize: rcp on 64 partitions, duplicated to 128 via a
            # partition-shifted DVE copy, then one mul
            rcp = wnorm.tile([P, CH], f32, tag="rn")
            scr = wnorm.tile([64, CH], f32, tag="ln")
            nc.vector.reciprocal_approx_accurate(out=rcp[0:64, :], in_=csum, scratch=scr)
            nc.vector.tensor_copy(rcp[64:128, :], rcp[0:64, :])
            nc.vector.tensor_mul(att_sb[h][:, cs_], attps, rcp)

        # ---- Phase C (o-proj) for one s-tile ----
        def emit_oproj_st(st):
            for mc in range(NCH):
                ops = p_acc.tile([P, CH], f32, tag="acc")
                for h in range(G):
                    nc.tensor.matmul(
                        ops, lhsT=att_sb[h][:, st * P:(st + 1) * P],
                        rhs=wo_sb[:, h, mc * CH:(mc + 1) * CH],
                        start=(h == 0), stop=(h == G - 1),
                    )
                osb = work.tile([P, CH], f32, tag="osb")
                nc.vector.tensor_copy(osb, ops)
                nc.sync.dma_start(
                    out=d_out[st * P:(st + 1) * P, mc * CH:(mc + 1) * CH], in_=osb)

        # wavefront: attention heads of chunk c interleave with o-proj
        # s-tiles of the previous attention chunk (PE fill work during the
        # ACT-bound exp stretches); cheap chunk 0 last for a short tail.
        # x-chunk DMAs are issued one phase ahead of use.
        emit_A(0)
        emit_A(1)
        for h in range(G):
            emit_att_head(1, h)
        emit_A(2)
        for h in range(G):
            emit_att_head(2, h)
            emit_oproj_st(4 * 1 + h)
        emit_A(3)
        for h in range(G):
            emit_att_head(3, h)
            emit_oproj_st(4 * 2 + h)
        for h in range(G):
            emit_att_head(0, h)
            emit_oproj_st(4 * 3 + h)
        for st in range(4):
            emit_oproj_st(st)

    # Pin every activation to the one table set that contains all functions
    # we use (exp/ln/copy/square), so the ACT engine never swaps tables.
    # Indices must stay aligned with act_info.json, so other sets are kept
    # in place but emptied (the pass then can't pick them).
    from concourse import bacc as bacc_mod
    orig_tables = bacc_mod.get_activation_tables
    target = "natural_log_exp_and_others"

    def unified_tables(arch):
        t = orig_tables(arch)
        assert target in t
        return {k: (v if k == target else set()) for k, v in t.items()}

    bacc_mod.get_activation_tables = unified_tables
    try:
        nc.compile()
    finally:
        bacc_mod.get_activation_tables = orig_tables
    return nc


def _get_nc():
    if "nc" not in _CACHE:
        _CACHE["nc"] = _build_nc()
    return _CACHE["nc"]


def _rope_tables():
    inv_ts = THETA ** (-np.arange(HD // 2, dtype=np.float64) / (HD // 2))
    ang = np.arange(S, dtype=np.float64)[None, :] * inv_ts[:, None]  # [64, S]
    cos64 = np.cos(ang)
    sin64 = np.sin(ang)
    cos_t = np.concatenate([cos64, cos64], 0).astype(np.float32)
    # rotate-then-multiply signs: top rows get -sin, bottom +sin
    sin_t = np.concatenate([-sin64, sin64], 0).astype(np.float32)
    return cos_t, sin_t


def kernel(x, wq, wk, wv, wo, q_scale, k_scale):
    bf = ml_dtypes.bfloat16
    x = np.asarray(x, np.float32)
    wq = np.asarray(wq, np.float32)
    wk = np.asarray(wk, np.float32)
    wv = np.asarray(wv, np.float32)
    wo = np.asarray(wo, np.float32)
    q_scale = np.asarray(q_scale, np.float32)
    k_scale = np.asarray(k_scale, np.float32)

    from concourse.bass_utils import run_bass_kernel_spmd

    nc = _get_nc()
    cos_t, sin_t = _rope_tables()
    half = P // 2
    psw = np.zeros((P, P), np.float32)
    psw[np.arange(half) + half, np.arange(half)] = 1.0
    psw[np.arange(half), np.arange(half) + half] = 1.0
    tri = (np.arange(P)[None, :] >= np.arange(P)[:, None]).astype(np.float32)
    trineg = np.where(np.arange(P)[None, :] >= np.arange(P)[:, None], 0.0, -50.0
                      ).astype(np.float32)

    in_maps = []
    for core in range(8):
        b, g = divmod(core, 4)
        in_maps.append({
            "xt": np.ascontiguousarray(
                x[b].T.reshape(KT, P, NCH, CH).transpose(2, 1, 0, 3)).astype(bf),
            "wq4": np.ascontiguousarray(
                wq[:, 4 * g:4 * g + 4, :].reshape(KT, P, G, HD).transpose(1, 0, 2, 3)).astype(bf),
            "wk1": np.ascontiguousarray(
                wk[:, g, :].reshape(KT, P, HD).transpose(1, 0, 2)).astype(bf),
            "wv1": np.ascontiguousarray(
                wv[:, g, :].reshape(KT, P, HD).transpose(1, 0, 2)).astype(bf),
            "wo4": np.ascontiguousarray(np.transpose(wo[4 * g:4 * g + 4], (1, 0, 2))).astype(bf),
            "qsc": q_scale.reshape(HD, 1),
            "ksc": k_scale.reshape(HD, 1),
            "cos_t": cos_t,
            "sin_t": sin_t,
            "psw": psw.astype(bf),
            "tri": tri.astype(bf),
            "trineg": trineg.astype(bf),
        })

    res = run_bass_kernel_spmd(nc, in_maps, list(range(8)), **_RUN_KWARGS)
    _CACHE["last_res"] = res
    out = np.zeros((B, S, DM), np.float32)
    for core in range(8):
        out[core // 4] += res.results[core]["o_part"]
    return out
